# revision 7
# baseline (speedup 1.0000x reference)
"""Bass/Trainium2 kernel for nn_ClusteringLayer (vq_codebook).

q = rownorm(1 / (1 + ||x - c||^2))   (ALPHA = 1 -> the power term is exactly 1)

Sharding: data-parallel over the sample axis across 8 NeuronCores; the
[K, D] centroid matrix is replicated.  Row normalization is per-sample so
no collectives are needed.

Per-core algorithm (x_s: [8192, 512] f32, clusters: [1024, 512] f32):
  TensorE (bf16): psum = x . c^T - (||c||^2 + 1)/2
      4 K=128 chunks of the D contraction  +  one K=2 "augmented" chunk:
      a ones[2,128] stationary against [c_hi; c_lo] (hi/lo bf16 split of
      -(||c||^2+1)/2) so the cluster constant rides the GEMM accumulation.
  ScalarE: t = Ln(-2*psum + bias)  with per-partition bias = 1 + ||x||^2
           q_u = Exp(-t)           with accum_out = per-row sum S (free)
  VectorE: bias via tensor_tensor_reduce(xb*xb, init=1.0);
           rinv = 1/S (bit-exact); q = q_u * rinv  (fp32 2x mode)
  x path: gpsimd casting DMA (DRAM f32 -> SBUF bf16), then xbar DMA
          transpose to put D on partitions for the GEMM.
"""

import os

import numpy as np

import bass_rust
import concourse.bass as bass
import concourse.mybir as mybir
import concourse.tile as tile
from concourse.bass_utils import run_bass_kernel_spmd

F32 = mybir.dt.float32
BF16 = mybir.dt.bfloat16

N_CORES = 8
N = 65536
D = 512
K = 1024
NS = N // N_CORES  # samples per core
P = 128
NCH = D // P  # 4 contraction chunks of 128
MT = NS // P  # 64 sample tiles per core


def build_kernel(fix_for_walrus: bool = True):
    nc = bass.Bass(
        "TRN2",
        target_bir_lowering=False,
        debug=False,
        num_devices=N_CORES,
    )
    x = nc.dram_tensor("x", [NS, D], F32, kind="ExternalInput").ap()
    clusters = nc.dram_tensor("clusters", [K, D], F32, kind="ExternalInput").ap()
    q = nc.dram_tensor("q", [NS, K], F32, kind="ExternalOutput").ap()

    with tile.TileContext(nc) as tc:
        _body(tc, q, x, clusters)
    if fix_for_walrus:
        _fix_bir_for_walrus(nc)
    return nc


# The installed walrus build rejects two emissions of this bass/tile version:
#   1. InstISA EVENT_SEMAPHORE_RANGE_CLEAR (opcode 176)  -> "ISA wrong length"
#   2. >1 sync wait on one instruction                    -> "Too many sync waits"
# Rewrite the BIR: split multi-waits into standalone EventSemaphore waits, and
# replace the tile-end range clear with explicit per-semaphore decrements of
# each semaphore's statically-known net increment (so the NEFF stays
# re-executable).
_MODE_SIGN = {"sem-inc": 1, "sem-add-imm": 1, "sem-dec": -1, "sem-sub-imm": -1}


def _fix_bir_for_walrus(nc):
    net = {}
    for f in nc.m.functions:
        for bb in f.blocks:
            for inst in bb.instructions:
                si = inst.sync_info
                if not si:
                    continue
                for u in si.on_update:
                    sign = _MODE_SIGN[u.update_mode]  # KeyError on unknown mode
                    net[u.id] = net.get(u.id, 0) + sign * u.update_value

    n_fix = 0
    for f in nc.m.functions:
        for bb in f.blocks:
            new_list = []
            changed = False
            for inst in bb.instructions:
                si = inst.sync_info
                if si and len(si.on_wait) > 1:
                    for wt in list(si.on_wait)[:-1]:
                        es = mybir.InstEventSemaphore(
                            name=f"I-fixw{n_fix}", engine=inst.engine, ins=[], outs=[]
                        )
                        es.sync_info = bass_rust.SyncInfo(on_wait=[wt], on_update=[])
                        new_list.append(es)
                        n_fix += 1
                    inst.sync_info = bass_rust.SyncInfo(
                        on_wait=[list(si.on_wait)[-1]], on_update=list(si.on_update)
                    )
                    changed = True
                if isinstance(inst, mybir.InstISA) and inst.isa_opcode == 176:
                    lo = inst.ant_dict["range_first"]
                    hi = inst.ant_dict["range_last"]
                    for sid in range(lo, hi + 1):
                        v = net.get(sid, 0)
                        if v:
                            es = mybir.InstEventSemaphore(
                                name=f"I-fixc{n_fix}",
                                engine=inst.engine,
                                ins=[],
                                outs=[],
                            )
                            u0 = bass_rust.SyncUpdate(
                                sync_type="semaphore",
                                id=sid,
                                update_mode="sem-sub-imm" if v > 0 else "sem-add-imm",
                                update_value=abs(v),
                            )
                            es.sync_info = bass_rust.SyncInfo(on_wait=[], on_update=[u0])
                            new_list.append(es)
                            n_fix += 1
                    changed = True
                    continue  # drop the range-clear itself
                new_list.append(inst)
            if changed:
                bb.instructions = new_list


def _body(tc: tile.TileContext, q: bass.AP, x: bass.AP, clusters: bass.AP):
    nc = tc.nc
    mult = mybir.AluOpType.mult
    add = mybir.AluOpType.add
    subtract = mybir.AluOpType.subtract
    Ln = mybir.ActivationFunctionType.Ln
    Exp = mybir.ActivationFunctionType.Exp

    with (
        tc.tile_pool(name="const", bufs=1) as const,
        tc.tile_pool(name="work", bufs=3) as work,
        tc.tile_pool(name="psum", bufs=2, space="PSUM") as psum,
    ):
        # ---------------- cluster setup (once per core) ----------------
        # clusters [1024, 512] -> 8 groups of 128 on partitions
        c_f32 = const.tile([P, 8, D], F32)
        nc.sync.dma_start(
            out=c_f32, in_=clusters.rearrange("(g p) d -> p g d", p=P)
        )
        c_bf = const.tile([P, 8, D], BF16)
        nc.vector.tensor_copy(out=c_bf, in_=c_f32)

        # ceT [128 d, 4 chunk, 1024 cluster]: ceT[p, j, k] = c[k, j*128+p]
        ceT = const.tile([P, NCH, K], BF16)
        for g in range(8):
            for j in range(NCH):
                nc.sync.dma_start_transpose(
                    ceT[:, j, g * P : (g + 1) * P],
                    c_bf[:, g, j * P : (j + 1) * P],
                )

        # c_sq row [1, 1024] via ones-matmul over the squared transposed tiles
        ceT_sq = const.tile([P, NCH, K], BF16)
        nc.vector.tensor_tensor(out=ceT_sq, in0=ceT, in1=ceT, op=mult)
        ones_col = const.tile([P, 1], BF16)
        nc.vector.memset(ones_col, 1.0)
        with tc.tile_pool(name="psum_setup", bufs=1, space="PSUM") as psum_setup:
            csq_ps = psum_setup.tile([1, K], F32)
            for j in range(NCH):
                for h in range(2):
                    sl = slice(h * 512, (h + 1) * 512)
                    nc.tensor.matmul(
                        out=csq_ps[:, sl],
                        lhsT=ones_col,
                        rhs=ceT_sq[:, j, sl],
                        start=(j == 0),
                        stop=(j == NCH - 1),
                    )
            # vrow = -(c_sq + 1)/2, split hi/lo into two bf16 rows
            vrow = const.tile([1, K], F32)
            nc.vector.tensor_scalar(
                out=vrow, in0=csq_ps, scalar1=-0.5, scalar2=-0.5, op0=mult, op1=add
            )
        ce_hi_p0 = const.tile([1, K], BF16)
        nc.vector.tensor_copy(out=ce_hi_p0, in_=vrow)
        resid = const.tile([1, K], F32)
        nc.vector.tensor_tensor(out=resid, in0=vrow, in1=ce_hi_p0, op=subtract)
        ce_lo_p0 = const.tile([1, K], BF16)
        nc.vector.tensor_copy(out=ce_lo_p0, in_=resid)
        ce_aug = const.tile([2, K], BF16)
        nc.sync.dma_start(out=ce_aug[0:1, :], in_=ce_hi_p0)
        nc.sync.dma_start(out=ce_aug[1:2, :], in_=ce_lo_p0)
        ones2 = const.tile([2, P], BF16)
        nc.vector.memset(ones2, 1.0)

        # ---------------- main loop over 64 sample tiles ----------------
        x_r = x.rearrange("(mt p) d -> mt p d", p=P)
        q_r = q.rearrange("(mt p) k -> mt p k", p=P)

        for mt in range(MT):
            # casting DMA: DRAM f32 -> SBUF bf16 (gpsimd SWDGE can cast)
            xb = work.tile([P, D], BF16, tag="xb")
            nc.gpsimd.dma_start(out=xb, in_=x_r[mt])

            # xT[p, j, s] = x[s, j*128+p] via xbar transpose
            xT = work.tile([P, NCH, P], BF16, tag="xT")
            nc.sync.dma_start_transpose(xT, xb)

            # bias = sum(x^2) per sample; the "+1" rides the augmented
            # cluster chunk (c_sq + 1).  scalar_tensor_tensor is used (not
            # tensor_tensor_reduce) because the latter's ISA encoding is
            # rejected by the installed walrus.
            xsq = work.tile([P, 1], F32, tag="xsq")
            sq_scratch = work.tile([P, D], F32, tag="sqg")
            nc.vector.scalar_tensor_tensor(
                out=sq_scratch,
                in0=xb,
                scalar=1.0,
                in1=xb,
                op0=mybir.AluOpType.bypass,
                op1=mult,
                accum_out=xsq,
            )

            # psum = x.c^T - (c_sq+1)/2
            ps = psum.tile([P, K], F32, tag="ps")
            for j in range(NCH):
                for h in range(2):
                    sl = slice(h * 512, (h + 1) * 512)
                    nc.tensor.matmul(
                        out=ps[:, sl],
                        lhsT=xT[:, j, :],
                        rhs=ceT[:, j, sl],
                        start=(j == 0),
                        stop=False,
                    )
            for h in range(2):
                sl = slice(h * 512, (h + 1) * 512)
                nc.tensor.matmul(
                    out=ps[:, sl],
                    lhsT=ones2,
                    rhs=ce_aug[:, sl],
                    start=False,
                    stop=True,
                )

            # t = Ln(-2*psum + (1+||x||^2))  ;  q_u = Exp(-t), S = row-sum
            t_t = work.tile([P, K], F32, tag="t")
            nc.scalar.activation(out=t_t, in_=ps, func=Ln, bias=xsq, scale=-2.0)
            qu = work.tile([P, K], F32, tag="qu")
            rowsum = work.tile([P, 1], F32, tag="rs")
            nc.scalar.activation(
                out=qu, in_=t_t, func=Exp, scale=-1.0, accum_out=rowsum
            )

            rinv = work.tile([P, 1], F32, tag="ri")
            nc.vector.reciprocal(out=rinv, in_=rowsum)
            qf = work.tile([P, K], F32, tag="qf")
            nc.vector.tensor_scalar_mul(out=qf, in0=qu, scalar1=rinv)

            nc.sync.dma_start(out=q_r[mt], in_=qf)


_BUILT = None


def _get_built():
    global _BUILT
    if _BUILT is None:
        _BUILT = build_kernel()
    return _BUILT


def _install_ntff_shim():
    """The agent image's `antenv` lacks `axon_hooks`, so trace=True under
    axon crashes on import.  Provide the missing glue module and register
    the boot shim's ctypes-based NTFF hook (dev-time profiling only)."""
    import sys
    import types

    if "antenv.axon_hooks" in sys.modules:
        return
    mod = types.ModuleType("antenv.axon_hooks")
    mod._hook = None

    def set_axon_ntff_profile_hook(h):
        mod._hook = h

    def get_axon_ntff_profile_hook():
        return mod._hook

    mod.set_axon_ntff_profile_hook = set_axon_ntff_profile_hook
    mod.get_axon_ntff_profile_hook = get_axon_ntff_profile_hook
    sys.modules["antenv.axon_hooks"] = mod
    try:
        from trn_agent_boot.trn_boot import _ntff_profile_via_ctypes

        mod._hook = _ntff_profile_via_ctypes("/opt/axon/libaxon_pjrt.so")
    except Exception as e:
        print(f"NTFF shim: hook unavailable ({e}); tracing will be skipped")


def run(inputs: dict, trace: bool = False):
    x = np.ascontiguousarray(np.asarray(inputs["x"], dtype=np.float32))
    clusters = np.ascontiguousarray(np.asarray(inputs["clusters"], dtype=np.float32))
    assert x.shape == (N, D) and clusters.shape == (K, D)

    if trace:
        _install_ntff_shim()
    nc = _get_built()
    in_maps = [
        {
            "x": np.ascontiguousarray(x[i * NS : (i + 1) * NS]),
            "clusters": clusters,
        }
        for i in range(N_CORES)
    ]
    res = run_bass_kernel_spmd(
        nc,
        in_maps,
        core_ids=list(range(N_CORES)),
        trace=trace,
    )
    out = np.concatenate([res.results[i]["q"] for i in range(N_CORES)], axis=0)
    return out, res


def kernel(**inputs) -> np.ndarray:
    out, _ = run(inputs, trace=bool(int(os.environ.get("KERNEL_TRACE", "0"))))
    return out
